# revision 27
# baseline (speedup 1.0000x reference)
"""Trainium2 Bass kernel for a pre-norm transformer decoder layer.

Sharding: 8 cores = 4 batches x 2 sequence-stripe halves.
Core c handles batch b=c//2 and the 1024 queries q with (q mod 512)//256 == c%2
(alternating 256-wide stripes -> causal-load-balanced and the per-core
program is identical across cores; only input data differs).

On-chip dataflow is feature-major ("transposed"): activations live as
[D, tokens]; every matmul contracts over the partition dim with zero on-chip
transposes (host pre-transposes x and all weights, output transposed back).

v2: the six weight GEMMs (Q/K/V/O proj, fc1, fc2) run in fp8-e4m3 with
MatmulPerfMode.DoubleRow -- two 128-chunks of the contraction dim per
instruction (out = W0^T@X0 + W1^T@X1), i.e. 2x PE throughput.  Weights are
host-quantized to fp8 after a power-of-2 scale (x32 / x64); the de-scale is
folded into the PSUM->SBUF activation readout.  LN outputs (xn1/xn2) and the
normalized attention context are stored fp8; scores/attn@V matmuls stay bf16
(their stationary operand changes every matmul, so DoubleRow would be
LDWEIGHTS-bound).  The attention residual y1 and context stay in SBUF (no
DRAM round-trip); the residual base reads the raw bf16 own-token tile.
LayerNorm statistics use bf16 ones-matmuls; mean/rstd rows broadcast across
partitions via K=1 outer-product matmuls (float32r).  Causal softmax skips
max-subtraction (scores are O(1) here) and applies a multiplicative 0/1
bf16 mask after exp; the softmax denominator accumulates in its own PSUM
bank alongside attn@V.  PSUM accumulation groups are strictly
bank-exclusive (a group's first matmul zeroes its whole bank).
"""

import numpy as np
import ml_dtypes

import concourse.bass as bass
import concourse.tile as tile
import concourse.mybir as mybir
from concourse.bass_utils import run_bass_kernel_spmd

FP32 = mybir.dt.float32
F32R = mybir.dt.float32r
BF16 = mybir.dt.bfloat16
FP8 = mybir.dt.float8e4
AOP = mybir.AluOpType
ACT = mybir.ActivationFunctionType
DR = mybir.MatmulPerfMode.DoubleRow
EPS = 1e-5
WS = 32.0     # host scale for W_q/W_k/W_v/W_o/fc1 before fp8 quantization
WS2 = 64.0    # host scale for fc2


def _split_drain_waits(nc, max_waits=1):
    """walrus here rejects >max_waits sync waits per instruction; split
    extras onto preceding single-wait NoOps on the same engine."""
    for f in nc.m.functions:
        for bb in f.blocks:
            insts = list(bb.instructions)
            out, changed = [], False
            for inst in insts:
                si = inst.sync_info
                if si is not None and len(si.on_wait) > max_waits:
                    waits = list(si.on_wait)
                    for j, w in enumerate(waits[:-max_waits]):
                        out.append(mybir.InstNoOp(
                            name=f"{inst.name}_sw{j}", ins=[], outs=[],
                            engine=inst.engine,
                            sync_info=mybir.SyncInfo(on_wait=[w],
                                                     on_update=[])))
                    inst.sync_info = mybir.SyncInfo(
                        on_wait=waits[-max_waits:],
                        on_update=list(si.on_update))
                    changed = True
                out.append(inst)
            if changed:
                bb.instructions = out


def build_decoder_nc(S=2048, D=1024, F=4096, apply_ln_affine=False,
                     debug=False, surgery=True, repeat=1):
    """Single-core Bass program (per-core shapes)."""
    DC = D // 128
    DP = DC // 2          # contraction pair count for DoubleRow
    FC = F // 128
    FP = FC // 2
    OWN = S // 2
    P = S // 512
    NKT = S // 128
    TS = 512
    NT = S // TS
    TSO = min(TS, OWN)
    NTO = OWN // TSO
    scale_q = 1.0 / float(np.sqrt(D))

    nc = bass.Bass()

    xTbf = nc.dram_tensor("xTbf", [D, S], BF16, kind="ExternalInput")
    xTobf = nc.dram_tensor("xTobf", [D, OWN], BF16, kind="ExternalInput")
    mask01 = nc.dram_tensor("mask01", [512, 256], BF16, kind="ExternalInput")
    wqT = nc.dram_tensor("wqT", [D, D], FP8, kind="ExternalInput")
    wkT = nc.dram_tensor("wkT", [D, D], FP8, kind="ExternalInput")
    wvT = nc.dram_tensor("wvT", [D, D], FP8, kind="ExternalInput")
    woT = nc.dram_tensor("woT", [D, D], FP8, kind="ExternalInput")
    fc1T = nc.dram_tensor("fc1T", [D, F], FP8, kind="ExternalInput")
    fc2T = nc.dram_tensor("fc2T", [F, D], FP8, kind="ExternalInput")
    fc1b = nc.dram_tensor("fc1b", [F], FP32, kind="ExternalInput")
    fc2b = nc.dram_tensor("fc2b", [D], FP32, kind="ExternalInput")
    lnp = None
    if apply_ln_affine:
        lnp = nc.dram_tensor("lnp", [4, D], FP32, kind="ExternalInput")
    outT = nc.dram_tensor("outT", [D, OWN], FP32, kind="ExternalOutput")

    xTbf_r = xTbf.rearrange("(c p) s -> p c s", p=128)
    xTobf_r = xTobf.rearrange("(c p) s -> p c s", p=128)
    mask_r = mask01.rearrange("(j p) t -> p j t", p=128)
    wqT_r = wqT.rearrange("(c p) e -> p c e", p=128)
    wkT_r = wkT.rearrange("(c p) e -> p c e", p=128)
    wvT_r = wvT.rearrange("(c p) e -> p c e", p=128)
    woT_r = woT.rearrange("(c p) e -> p c e", p=128)
    fc1T_r = fc1T.rearrange("(c p) f -> p c f", p=128)
    fc2T_r = fc2T.rearrange("(c p) d -> p c d", p=128)
    fc1b_r = fc1b.rearrange("(c p) -> p c", p=128)
    fc2b_r = fc2b.rearrange("(c p) -> p c", p=128)
    outT_r = outT.rearrange("(c p) s -> p c s", p=128)

    with tile.TileContext(nc) as tc:
        with (
            tc.tile_pool(name="consts", bufs=1) as consts,
            tc.tile_pool(name="work", bufs=2) as work,
        ):
            ones_col = consts.tile([128, 1], BF16, tag="ones_col")
            nc.vector.memset(ones_col, 1.0)
            ones2 = consts.tile([128, 2, 1], FP8, tag="ones2")
            nc.vector.memset(ones2, 1.0)
            ones_row = consts.tile([1, 128], F32R, tag="ones_row")
            ones_row_f = consts.tile([1, 128], FP32, tag="ones_row_f")
            nc.vector.memset(ones_row_f, 1.0)
            nc.vector.tensor_copy(ones_row, ones_row_f)
            eps_t = consts.tile([1, 1], FP32, tag="eps")
            nc.vector.memset(eps_t, EPS)
            fc1b_t = consts.tile([128, FC], FP32, tag="fc1b")
            nc.sync.dma_start(fc1b_t, fc1b_r)
            fc2b_t = consts.tile([128, DC], FP32, tag="fc2b")
            nc.sync.dma_start(fc2b_t, fc2b_r)
            maskt = consts.tile([128, 4, 256], BF16, tag="mask")
            nc.sync.dma_start(maskt, mask_r)
            lnp_t = None
            if apply_ln_affine:
                lnp_t = consts.tile([128, 4, DC], FP32, tag="lnp")
                nc.sync.dma_start(
                    lnp_t, lnp.rearrange("g (c p) -> p g c", p=128))

            def ln_tile(src3, dst3, src_ti, dst_ti, ts, ln_ps, bc_ps,
                        g_idx, b_idx):
                """LayerNorm stats from bf16 src3 tile; apply writes dst3
                (any dtype, usually fp8).  src3/dst3 may tile their token
                axes differently (src_ti vs dst_ti)."""
                ssl = slice(src_ti * ts, (src_ti + 1) * ts)
                sl = slice(dst_ti * ts, (dst_ti + 1) * ts)
                ps_sx = ln_ps.tile([1, ts], FP32, tag="ps_sx")
                ps_sq = ln_ps.tile([1, ts], FP32, tag="ps_sq")
                for dc in range(DC):
                    src = src3[:, dc, ssl]
                    sq = work.tile([128, ts], BF16, tag="lnsq")
                    nc.scalar.square(sq, src)
                    nc.tensor.matmul(ps_sx, ones_col, src,
                                     start=(dc == 0), stop=(dc == DC - 1))
                    nc.tensor.matmul(ps_sq, ones_col, sq,
                                     start=(dc == 0), stop=(dc == DC - 1))
                mu = work.tile([1, ts], F32R, tag="r_mu")
                rs = work.tile([1, ts], F32R, tag="r_rs")
                msq = work.tile([1, ts], F32R, tag="r_msq")
                nc.scalar.activation(mu, ps_sx, ACT.Copy, scale=1.0 / D)
                nc.scalar.activation(rs, ps_sq, ACT.Copy, scale=1.0 / D)
                nc.vector.tensor_mul(msq, mu, mu)
                nc.vector.tensor_sub(rs, rs, msq)
                nc.scalar.activation(rs, rs, ACT.Sqrt, bias=eps_t)
                with nc.allow_low_precision(reason="rstd row f32r"):
                    nc.vector.reciprocal(rs, rs)
                mb = bc_ps.tile([128, ts], FP32, tag="bc")
                nc.tensor.matmul(mb, ones_row, mu)
                rb = bc_ps.tile([128, ts], FP32, tag="bc")
                nc.tensor.matmul(rb, ones_row, rs)
                # the apply reads the broadcast rows straight from PSUM
                for dc in range(DC):
                    t1 = work.tile([128, ts], FP32, tag="lnt1", bufs=1)
                    nc.vector.tensor_sub(t1, src3[:, dc, ssl], mb)
                    if apply_ln_affine:
                        t2 = work.tile([128, ts], FP32, tag="lnt2")
                        nc.vector.scalar_tensor_tensor(
                            t2, t1,
                            lnp_t[:, g_idx, dc:dc + 1], rb,
                            AOP.mult, AOP.mult)
                        nc.vector.tensor_scalar_add(
                            dst3[:, dc, sl], t2,
                            lnp_t[:, b_idx, dc:dc + 1])
                    else:
                        nc.vector.tensor_mul(dst3[:, dc, sl], t1, rb)

            for _rep in range(repeat):
              # xn2 (fp8 LN2 output) + y1 (bf16 attn residual) persist
              # across attention -> FFN.
              with tc.tile_pool(name="persist", bufs=1) as pers:
                ctxn = pers.tile([128, DC, OWN], FP8, tag="ctxn")
                xn2 = pers.tile([128, DC, OWN], FP8, tag="xn2")
                y1 = pers.tile([128, DC, OWN], BF16, tag="y1")
                h = pers.tile([128, FC, OWN], FP8, tag="h")
                with tc.tile_pool(name="L1", bufs=1) as L1:
                    KT = L1.tile([128, DC, S], BF16, tag="KT")
                    VT = L1.tile([128, NKT, D], BF16, tag="VT")
                    QT = L1.tile([128, DC, OWN], BF16, tag="QT")

                    with (
                        tc.tile_pool(name="lnpsA", bufs=1,
                                     space="PSUM") as lnpsA,
                        tc.tile_pool(name="bcpsA", bufs=2,
                                     space="PSUM") as bcpsA,
                        tc.tile_pool(name="projps", bufs=4,
                                     space="PSUM") as pps,
                        tc.tile_pool(name="wpool", bufs=2) as wp,
                    ):
                        # LN1-full + K + V, pipelined per token tile.
                        # Raw x lives per-tile (double-buffered); only the
                        # fp8 LN output is kept for the full sequence.
                        with (
                            tc.tile_pool(name="xn1p", bufs=2) as xn1p,
                            tc.tile_pool(name="xn1np", bufs=2) as xn1np,
                        ):
                            WK = WV = None

                            def kv_proj(ti, xn1n):
                                sl = slice(ti * TS, (ti + 1) * TS)
                                for ec in range(DC):
                                    ps = pps.tile([128, TS], FP32,
                                                  tag="pps")
                                    for dp in range(DP):
                                        nc.tensor.matmul(
                                            ps,
                                            WK[:, 2 * dp:2 * dp + 2,
                                               ec * 128:(ec + 1) * 128],
                                            xn1n[:, 2 * dp:2 * dp + 2, :],
                                            start=(dp == 0),
                                            stop=(dp == DP - 1),
                                            perf_mode=DR)
                                    nc.scalar.activation(KT[:, ec, sl], ps,
                                                         ACT.Copy,
                                                         scale=1.0 / WS)
                                for tk in range(4):
                                    tcn = ti * 4 + tk
                                    for eh in range(D // 512):
                                        esl = slice(eh * 512,
                                                    (eh + 1) * 512)
                                        ps = pps.tile([128, 512], FP32,
                                                      tag="pps")
                                        for dp in range(DP):
                                            nc.tensor.matmul(
                                                ps,
                                                xn1n[:, 2 * dp:2 * dp + 2,
                                                     tk * 128:
                                                     (tk + 1) * 128],
                                                WV[:, 2 * dp:2 * dp + 2,
                                                   esl],
                                                start=(dp == 0),
                                                stop=(dp == DP - 1),
                                                perf_mode=DR)
                                        nc.scalar.activation(
                                            VT[:, tcn, esl], ps, ACT.Copy,
                                            scale=1.0 / WS)

                            prev = None
                            for ti in range(NT):
                                sl = slice(ti * TS, (ti + 1) * TS)
                                xn1 = xn1p.tile([128, DC, TS], BF16,
                                                tag="xn1")
                                for dc in range(DC):
                                    nc.sync.dma_start(xn1[:, dc, :],
                                                      xTbf_r[:, dc, sl])
                                if ti == 0:
                                    # weight DMAs queue behind the first x
                                    # tile so LN can start immediately
                                    WK = wp.tile([128, DC, D], FP8, tag="w")
                                    for dc in range(DC):
                                        nc.sync.dma_start(WK[:, dc, :],
                                                          wkT_r[:, dc, :])
                                    WV = wp.tile([128, DC, D], FP8, tag="w")
                                    for dc in range(DC):
                                        nc.sync.dma_start(WV[:, dc, :],
                                                          wvT_r[:, dc, :])
                                xn1n = xn1np.tile([128, DC, TS], FP8,
                                                  tag="xn1n")
                                ln_tile(xn1, xn1n, 0, 0, TS, lnpsA, bcpsA,
                                        0, 1)
                                if prev is not None:
                                    kv_proj(*prev)
                                prev = (ti, xn1n)
                            kv_proj(*prev)
                        # LN1-own + Q, per token tile
                        WQ = wp.tile([128, DC, D], FP8, tag="w")
                        for dc in range(DC):
                            nc.sync.dma_start(WQ[:, dc, :], wqT_r[:, dc, :])
                        with (
                            tc.tile_pool(name="xn1onp", bufs=1) as xn1onp,
                            tc.tile_pool(name="xop", bufs=2) as xop,
                        ):
                            xn1on = xn1onp.tile([128, DC, OWN], FP8,
                                                tag="xn1on")

                            def q_proj(tj):
                                tsl = slice(tj * TSO, (tj + 1) * TSO)
                                for ec in range(DC):
                                    ps = pps.tile([128, TSO], FP32,
                                                  tag="pps")
                                    for dp in range(DP):
                                        nc.tensor.matmul(
                                            ps,
                                            WQ[:, 2 * dp:2 * dp + 2,
                                               ec * 128:(ec + 1) * 128],
                                            xn1on[:, 2 * dp:2 * dp + 2,
                                                  tsl],
                                            start=(dp == 0),
                                            stop=(dp == DP - 1),
                                            perf_mode=DR)
                                    nc.scalar.activation(
                                        QT[:, ec, tsl], ps, ACT.Copy,
                                        scale=scale_q / WS)

                            for tj in range(NTO):
                                tsl = slice(tj * TSO, (tj + 1) * TSO)
                                xo = xop.tile([128, DC, TSO], BF16,
                                              tag="xo")
                                for dc in range(DC):
                                    nc.sync.dma_start(xo[:, dc, :],
                                                      xTobf_r[:, dc, tsl])
                                ln_tile(xo, xn1on, 0, tj, TSO, lnpsA,
                                        bcpsA, 0, 1)
                                if tj > 0:
                                    q_proj(tj - 1)
                            q_proj(NTO - 1)

                    # ---- attention: scores/softmax/attn@V (bf16) ----
                    with (
                        tc.tile_pool(name="s_ps", bufs=2,
                                     space="PSUM") as sps,
                        tc.tile_pool(name="av_ps", bufs=2,
                                     space="PSUM") as avp,
                        tc.tile_pool(name="dn_ps", bufs=1,
                                     space="PSUM") as dnp,
                        tc.tile_pool(name="bc1_ps", bufs=1,
                                     space="PSUM") as bc1,
                        tc.tile_pool(name="ptp", bufs=16) as ptp,
                        tc.tile_pool(name="pep", bufs=3) as pep,
                        tc.tile_pool(name="attw", bufs=2) as attw,
                        tc.tile_pool(name="denr", bufs=2) as denr,
                    ):
                        for p in range(P):
                            qsl = slice(p * 256, (p + 1) * 256)
                            nkt = (p + 1) * 4
                            dn = dnp.tile([1, 256], FP32, tag="dn")
                            pts = []
                            for kt in range(nkt):
                                s_ps = sps.tile([128, 256], FP32, tag="s")
                                for ec in range(DC):
                                    nc.tensor.matmul(
                                        s_ps,
                                        KT[:, ec, kt * 128:(kt + 1) * 128],
                                        QT[:, ec, qsl],
                                        start=(ec == 0), stop=(ec == DC - 1))
                                pt = ptp.tile([128, 256], BF16, tag="pt")
                                if kt // 4 == p:
                                    pe = pep.tile([128, 256], BF16, tag="pe")
                                    nc.scalar.activation(pe, s_ps, ACT.Exp)
                                    nc.vector.tensor_mul(
                                        pt, pe, maskt[:, kt % 4, :])
                                else:
                                    nc.scalar.activation(pt, s_ps, ACT.Exp)
                                nc.tensor.matmul(dn, ones_col, pt,
                                                 start=(kt == 0),
                                                 stop=(kt == nkt - 1))
                                pts.append(pt)
                            den = denr.tile([1, 256], F32R, tag="den")
                            nc.vector.tensor_copy(den, dn)
                            with nc.allow_low_precision(
                                    reason="softmax denom"):
                                nc.vector.reciprocal(den, den)
                            den_b = bc1.tile([128, 256], FP32, tag="bc1")
                            nc.tensor.matmul(den_b, ones_row, den)
                            den_sb = attw.tile([128, 256], FP32, tag="densb",
                                               bufs=1)
                            nc.scalar.activation(den_sb, den_b, ACT.Copy)
                            for dc in range(DC):
                                cps = avp.tile([128, 256], FP32, tag="av")
                                for kt in range(nkt):
                                    nc.tensor.matmul(
                                        cps,
                                        VT[:, kt, dc * 128:(dc + 1) * 128],
                                        pts[kt], start=(kt == 0),
                                        stop=(kt == nkt - 1))
                                # normalized context -> fp8 (attn-weighted
                                # average of v, O(1) range)
                                nc.vector.tensor_mul(ctxn[:, dc, qsl], cps,
                                                     den_sb)

# L1 (KT/VT/QT) closes here; O-proj + LN2 + FFN share one scope so
                # fc1 compute and weight DMAs overlap the LN2 tail.
                with (
                        tc.tile_pool(name="mm_ps", bufs=4,
                                     space="PSUM") as fps,
                        tc.tile_pool(name="ln2_ps", bufs=1,
                                     space="PSUM") as ln2ps,
                        tc.tile_pool(name="bc2_ps", bufs=2,
                                     space="PSUM") as bc2,
                        tc.tile_pool(name="wop", bufs=1) as wop,
                        tc.tile_pool(name="bigw", bufs=2) as bw,
                        tc.tile_pool(name="w2pool", bufs=1) as w2p,
                        tc.tile_pool(name="outp", bufs=3) as otp,
                ):
                        w2 = w2p.tile([128, FC, D], FP8, tag="w2")
                        WO = wop.tile([128, DC, D], FP8, tag="WO")
                        for dc in range(DC):
                            nc.sync.dma_start(WO[:, dc, :], woT_r[:, dc, :])
                        # ---- O-projection + residual + LN2 ----
                        for tj in range(NTO):
                            sl = slice(tj * TSO, (tj + 1) * TSO)
                            xo = bw.tile([128, DC, TSO], BF16, tag="xo")
                            for dc in range(DC):
                                nc.sync.dma_start(xo[:, dc, :],
                                                  xTobf_r[:, dc, sl])
                            for ec in range(DC):
                                ops_t = fps.tile([128, TSO], FP32,
                                                 tag="fps")
                                for dp in range(DP):
                                    nc.tensor.matmul(
                                        ops_t,
                                        WO[:, 2 * dp:2 * dp + 2,
                                           ec * 128:(ec + 1) * 128],
                                        ctxn[:, 2 * dp:2 * dp + 2, sl],
                                        start=(dp == 0), stop=(dp == DP - 1),
                                        perf_mode=DR)
                                # y1 = attn_out/WS + x: one DVE op reading
                                # the PSUM accumulator directly
                                nc.vector.scalar_tensor_tensor(
                                    y1[:, ec, sl], ops_t, 1.0 / WS,
                                    xo[:, ec, :], AOP.mult, AOP.add)
                        # LN2 pass (separate so the PE's O-proj of tile 1
                        # never waits on tile 0's DVE tail)
                        for tj in range(NTO):
                            ln_tile(y1, xn2, tj, tj, TSO, ln2ps, bc2, 2, 3)
                        FQ = FC // 4
                        for quar in range(4):
                            w1 = bw.tile([128, DC, FQ * 128], FP8,
                                         tag="bigw")
                            for dc in range(DC):
                                nc.sync.dma_start(
                                    w1[:, dc, :],
                                    fc1T_r[:, dc, quar * FQ * 128:
                                           (quar + 1) * FQ * 128])
                            for tj in range(NTO):
                                sl = slice(tj * TSO, (tj + 1) * TSO)
                                for fi in range(FQ):
                                    fc = quar * FQ + fi
                                    ps = fps.tile([128, TSO], FP32,
                                                  tag="fps")
                                    for dp in range(DP):
                                        nc.tensor.matmul(
                                            ps,
                                            w1[:, 2 * dp:2 * dp + 2,
                                               fi * 128:(fi + 1) * 128],
                                            xn2[:, 2 * dp:2 * dp + 2, sl],
                                            start=(dp == 0),
                                            stop=(dp == DP - 1),
                                            perf_mode=DR)
                                    nc.scalar.activation(
                                        h[:, fc, sl], ps, ACT.Relu,
                                        bias=fc1b_t[:, fc:fc + 1],
                                        scale=1.0 / WS)
                        for fi in range(FC):
                            nc.sync.dma_start(w2[:, fi, :], fc2T_r[:, fi, :])
                        # fc2: full contraction (16 DoubleRow MMs) per
                        # (dc, tj) PSUM group
                        for tj in range(NTO):
                            sl = slice(tj * TSO, (tj + 1) * TSO)
                            for dc in range(DC):
                                ps = fps.tile([128, TSO], FP32, tag="fps")
                                for fp_ in range(FP):
                                    nc.tensor.matmul(
                                        ps,
                                        w2[:, 2 * fp_:2 * fp_ + 2,
                                           dc * 128:(dc + 1) * 128],
                                        h[:, 2 * fp_:2 * fp_ + 2, sl],
                                        start=(fp_ == 0),
                                        stop=(fp_ == FP - 1),
                                        perf_mode=DR)
                                ot = otp.tile([128, TSO], FP32, tag="ot")
                                nc.vector.tensor_scalar(
                                    ot, ps, 1.0 / WS2,
                                    fc2b_t[:, dc:dc + 1],
                                    AOP.mult, AOP.add)
                                ot2 = otp.tile([128, TSO], FP32, tag="ot2")
                                nc.vector.tensor_add(ot2, ot, y1[:, dc, sl])
                                nc.sync.dma_start(outT_r[:, dc, sl], ot2)

    if surgery:
        _split_drain_waits(nc)
    return nc


# ---------------- host side ----------------

_NC_CACHE = {}


def _get_nc(S, D, F, apply_ln_affine, repeat=1):
    key = (S, D, F, apply_ln_affine, repeat)
    if key not in _NC_CACHE:
        _NC_CACHE[key] = build_decoder_nc(S, D, F, apply_ln_affine,
                                          repeat=repeat)
    return _NC_CACHE[key]


def _q8(a, scale):
    """Quantize to TRN fp8-e4m3 after power-of-2 scale (clip to +-240)."""
    f8 = mybir.dt.np(mybir.dt.float8e4)
    return np.clip(np.asarray(a, np.float32) * scale, -240.0,
                   240.0).astype(f8)


def make_in_maps(x, W_q, W_k, W_v, W_o, fc1_w, fc1_b, fc2_w, fc2_b,
                 ln1_g, ln1_b, ln2_g, ln2_b, apply_ln_affine):
    B, S, D = x.shape
    bf = ml_dtypes.bfloat16
    shared = {
        "wqT": _q8(W_q.T, WS),
        "wkT": _q8(W_k.T, WS),
        "wvT": _q8(W_v.T, WS),
        "woT": _q8(W_o.T, WS),
        "fc1T": _q8(fc1_w.T, WS),
        "fc2T": _q8(fc2_w.T, WS2),
        "fc1b": np.ascontiguousarray(fc1_b, dtype=np.float32),
        "fc2b": np.ascontiguousarray(fc2_b, dtype=np.float32),
    }
    if apply_ln_affine:
        shared["lnp"] = np.ascontiguousarray(
            np.stack([ln1_g, ln1_b, ln2_g, ln2_b]), dtype=np.float32)
    in_maps, stripes = [], []
    for c in range(2 * B):
        b, hh = c // 2, c % 2
        stripe = (np.arange(S) % 512) // 256 == hh
        stripes.append((b, stripe))
        xTb = np.ascontiguousarray(x[b].T, dtype=np.float32)
        m = np.zeros((512, 256), dtype=bf)
        tk = np.arange(512)[:, None]
        j = np.arange(256)[None, :]
        m[tk <= j + 256 * hh] = 1.0
        xTo = np.ascontiguousarray(xTb[:, stripe])
        in_maps.append(dict(shared,
                            xTbf=xTb.astype(bf),
                            xTobf=xTo.astype(bf),
                            mask01=m))
    return in_maps, stripes


def run_decoder(x, W_q, W_k, W_v, W_o, fc1_w, fc1_b, fc2_w, fc2_b,
                ln1_g, ln1_b, ln2_g, ln2_b, trace=False):
    x = np.asarray(x, dtype=np.float32)
    B, S, D = x.shape
    F = fc1_w.shape[0]
    apply_ln_affine = not (
        np.all(np.asarray(ln1_g) == 1.0) and np.all(np.asarray(ln1_b) == 0.0)
        and np.all(np.asarray(ln2_g) == 1.0)
        and np.all(np.asarray(ln2_b) == 0.0))
    nc = _get_nc(S, D, F, apply_ln_affine)
    in_maps, stripes = make_in_maps(
        x, np.asarray(W_q), np.asarray(W_k), np.asarray(W_v),
        np.asarray(W_o), np.asarray(fc1_w), np.asarray(fc1_b),
        np.asarray(fc2_w), np.asarray(fc2_b), np.asarray(ln1_g),
        np.asarray(ln1_b), np.asarray(ln2_g), np.asarray(ln2_b),
        apply_ln_affine)
    res = run_bass_kernel_spmd(nc, in_maps, core_ids=list(range(2 * B)),
                               trace=trace)
    out = np.empty((B, S, D), dtype=np.float32)
    for c in range(2 * B):
        b, stripe = stripes[c]
        out[b, stripe, :] = res.results[c]["outT"].T
    return out, res


def kernel(**inputs):
    out, _ = run_decoder(**inputs)
    return out


def _build_pjrt_fn(nc, in_maps):
    """Build a non-donating jitted executor + device-resident args."""
    import jax
    from jax.sharding import Mesh, PartitionSpec, NamedSharding
    from jax.experimental.shard_map import shard_map
    from concourse import bass2jax

    n_cores = len(in_maps)
    bass2jax.install_neuronx_cc_hook()
    partition_name = (nc.partition_id_tensor.name
                      if nc.partition_id_tensor else None)
    in_names, out_names, out_avals, zero_outs = [], [], [], []
    for alloc in nc.m.functions[0].allocations:
        if not isinstance(alloc, mybir.MemoryLocationSet):
            continue
        name = alloc.memorylocations[0].name
        if alloc.kind == "ExternalInput":
            if name != partition_name:
                in_names.append(name)
        elif alloc.kind == "ExternalOutput":
            shape = tuple(alloc.tensor_shape)
            dtype = mybir.dt.np(alloc.dtype)
            out_names.append(name)
            out_avals.append(jax.core.ShapedArray(shape, dtype))
            zero_outs.append(np.zeros(shape, dtype))
    n_params = len(in_names)
    in_names.extend(out_names)
    if partition_name is not None:
        in_names.append(partition_name)

    def _body(*args):
        operands = list(args)
        if partition_name is not None:
            operands.append(bass2jax.partition_id_tensor())
        return tuple(bass2jax._bass_exec_p.bind(
            *operands, out_avals=tuple(out_avals), in_names=tuple(in_names),
            out_names=tuple(out_names), lowering_input_output_aliases=(),
            sim_require_finite=True, sim_require_nnan=True, nc=nc))

    devices = jax.devices()[:n_cores]
    mesh = Mesh(np.asarray(devices), ("core",))
    fn = jax.jit(shard_map(
        _body, mesh=mesh,
        in_specs=(PartitionSpec("core"),) * (n_params + len(out_names)),
        out_specs=(PartitionSpec("core"),) * len(out_names),
        check_rep=False), keep_unused=True)
    sh = NamedSharding(mesh, PartitionSpec("core"))
    args = []
    for i in range(n_params):
        cat = np.concatenate([np.asarray(in_maps[c][in_names[i]])
                              for c in range(n_cores)], axis=0)
        args.append(jax.device_put(cat, sh))
    for z in zero_outs:
        args.append(jax.device_put(
            np.zeros((n_cores * z.shape[0], *z.shape[1:]), z.dtype), sh))
    return fn, args


def measure_body_ns(iters=4, n1=4, n2=16, r1=6, r2=10, **inputs):
    """Isolate per-execution NEFF body time from host/axon dispatch
    overhead: slope of async-pipelined executions, differenced between
    repeat=r1 and repeat=r2 NEFFs.  Both repeat counts are large enough
    that each execution is firmly device-bound (dispatch overhead and
    network jitter cancel in the difference).  Returns
    (body_ns, slope_r1_per_body_ns)."""
    import time
    import jax

    x = np.asarray(inputs["x"], dtype=np.float32)
    B, S, D = x.shape
    F = np.asarray(inputs["fc1_w"]).shape[0]
    in_maps, _ = make_in_maps(
        x, *[np.asarray(inputs[k]) for k in
             ("W_q", "W_k", "W_v", "W_o", "fc1_w", "fc1_b", "fc2_w", "fc2_b",
              "ln1_g", "ln1_b", "ln2_g", "ln2_b")], False)

    def slope(repeat):
        nc = _get_nc(S, D, F, False, repeat=repeat)
        fn, args = _build_pjrt_fn(nc, in_maps)
        o = fn(*args)
        jax.block_until_ready(o)
        ts = {}
        for N in (n1, n2):
            best = float("inf")
            for _ in range(iters):
                t0 = time.perf_counter()
                for _i in range(N):
                    o = fn(*args)
                jax.block_until_ready(o)
                best = min(best, time.perf_counter() - t0)
            ts[N] = best
        return (ts[n2] - ts[n1]) / (n2 - n1)

    s1 = slope(r1)
    s2 = slope(r2)
    return (s2 - s1) / (r2 - r1) * 1e9, s1 / r1 * 1e9
